# revision 22
# baseline (speedup 1.0000x reference)
"""BitLinear (packed +/-1 linear layer) Trainium2 kernel.

Math: out[b,o] = sum_k a[b,k]*w[o,k] + bias[o], where a/w are +/-1 values
bit-packed LSB-first into bytes (stored as int32 0..255).

Device strategy (8 NeuronCores, data-parallel over batch):
  - Each core gets B/8 = 1024 batch rows; the full weight matrix is
    replicated.
  - Host sends *transposed packed* uint8 tensors (k on partitions), so the
    device unpacks bits directly into the [K, *] layout the PE needs
    (contraction dim on partitions for both operands; the bit-interleaved
    k-order is consistent between A and W so the dot product is unchanged).
  - Unpack: one DVE tensor_scalar per (kp-tile, bit) moves bit i of every
    byte to bit position 6 and masks:  y = (x << (6-i)) & 0x40  (i=7 uses
    >> 1). Byte value 0x40 bitcast to fp8e4 reads as exactly 2.0, i.e.
    unpacked operands take values {0, 2.0} with no dtype-cast op (walrus
    forbids bitVec ops that cast). Ops run on uint16 views: both byte
    lanes' kept bits source from their own byte for these shifts, and the
    16-bit dtype enables the DVE 4x perf mode.
  - fp8e4 DoubleRow matmuls (256-deep contraction per instruction),
    activation tile stationary, N=512 per PSUM bank; psum = 4*M where
    M is the {0,1} binary dot.
  - Identity: with a = 2*alpha-1, w = 2*omega-1 (alpha,omega in {0,1}):
      out = 4*M - 2*rowsum(alpha) - 2*rowsum(omega) + K + bias
          = psum + r2[b] + c[o]
    where r2[b] = -2*popcount_rows(A), c[o] = bias + K - 2*popcount_rows(W)
    are cheap linear-time host precomputes (fp32-exact integers). The
    epilogue is one scalar_tensor_tensor per psum bank:
      out = (psum + r2_per_partition) + c_tile.

Everything is exact: products in {0,4}, fp32 PSUM accumulation of integers
<= 2^14, so the only rounding differences vs the fp32 reference are in the
final bias add (~1 ulp).

Performance model (measured via paired rep-delta microbenchmarks):
  - Pure DoubleRow matmul issue rate: ~216-220 ns per N=512 matmul
    (512 stream cycles at the ~2.33-2.40 GHz effective PE clock, plus
    ~1-3 cycles/instruction overhead; N=256 matmuls run at 108.6 ns,
    ratio 0.503, so the per-instruction overhead is negligible).
  - LDWEIGHTS is fully shadow-loaded: an oq-major variant with 4x more
    weight loads times identically, so DoubleRowSwInterleave or wider
    reuse buy nothing. Walrus rejects DR matmuls wider than one PSUM
    bank (512 f32), so instruction count cannot be reduced further.
  - Steady-state per-invocation device time == matmul stream time
    (1024 matmuls x ~216-220 ns ~= 221-225 us/core): the unpacking,
    epilogue, and DMA all hide under the PE stream; non-matmul exposed
    time is <1 us. This is ~99% of the fp8 DoubleRow roofline
    (524288 PE column-cycles = 218.5 us at 2.4 GHz, 157 TFLOP/s fp8
    peak); deeper packing (>2 bit-MACs/cell/cycle) is blocked by the
    fp8 operand mantissa and the 24-bit fp32 PSUM mantissa on every
    encoding scheme.
"""

import os
import sys

import numpy as np

for _p in ("/opt/trn_rl_repo", "/root/.axon_site/_ro/trn_rl_repo"):
    if os.path.isdir(_p) and _p not in sys.path:
        sys.path.append(_p)

BATCH = 8192
IN_FEATURES = 4096
OUT_FEATURES = 4096
PACKED_LEN = IN_FEATURES // 8  # 512
N_CORES = 8
P = 128

_NC_CACHE: dict = {}
LAST_RESULTS = None  # stash of the most recent BassKernelResults (for test.py)


def build_program(B, O, K, n_devices=N_CORES, o_half=2048, reps=1,
                  mm_reps=1, up_reps=1, out_bufs=3, stage_bufs=4,
                  psum_bufs=2, a2_bufs=1, w2_bufs=1, mm_width=512,
                  mm_order="k2", host_a=False, w_blk=False):
    """Emit the per-core Bass/Tile program. SPMD: same program every core.

    reps>1 repeats the whole compute body (identical writes) so test.py can
    measure pure device time as (T(reps=R) - T(reps=1)) / (R - 1).
    mm_reps / up_reps repeat only the matmul block (restarting PSUM
    accumulation, so the last rep wins) / only the unpack ops (idempotent)
    -- engine-rate microbenchmarks via the same delta method."""
    import concourse.bass as bass  # noqa: F401
    import concourse.mybir as mybir
    import concourse.tile as tile
    from concourse import bacc

    KP = K // 8  # packed k rows
    NT = KP // P  # kp tiles (4)
    NK2 = K // 256  # DoubleRow k-pair tiles (16)
    OH = min(O, o_half)  # o columns processed per outer phase
    NH = O // OH
    NOQ = OH // 512  # psum banks per phase
    NOQ2 = OH // mm_width  # matmul-width blocks per phase
    NB = B // P  # batch tiles
    assert KP % P == 0 and O % OH == 0 and OH % 512 == 0 and B % P == 0
    assert NK2 * 2 == NT * 8

    u8 = mybir.dt.uint8
    u16 = mybir.dt.uint16
    f32 = mybir.dt.float32
    fp8 = mybir.dt.float8e4
    shl = mybir.AluOpType.logical_shift_left
    shr = mybir.AluOpType.logical_shift_right
    band = mybir.AluOpType.bitwise_and
    add = mybir.AluOpType.add

    nc = bacc.Bacc(
        "TRN2",
        target_bir_lowering=False,
        debug=False,
        num_devices=n_devices,
    )

    if host_a:
        # host-unpacked activations: row (4t+q)*P+p, col j*B+m holds
        # 0x40*bit_{2q+j}(A[t*P+p, m]) -- DMAs straight into a2 tiles
        a2_d = nc.dram_tensor("a2h", [NK2 * P, 2 * B], u8, kind="ExternalInput").ap()
    else:
        at_d = nc.dram_tensor("at", [KP, B], u8, kind="ExternalInput").ap()
    wt_d = nc.dram_tensor("wt", [KP, O], u8, kind="ExternalInput").ap()
    c_d = nc.dram_tensor("c_rep", [P, O], f32, kind="ExternalInput").ap()
    r2_d = nc.dram_tensor("r2t", [P, NB], f32, kind="ExternalInput").ap()
    out_d = nc.dram_tensor("out", [B, O], f32, kind="ExternalOutput").ap()

    def unpack_ops(i):
        # Packed byte -> {0x00, 0x40} per byte lane for bit i: move the bit
        # to position 6 and mask (0x40 bitcast to fp8e4 reads as exactly
        # 2.0). Ops run on uint16 views (2 byte-lanes per element): for
        # shifts <= 6 left / 1 right, each kept bit (6 and 14) sources from
        # its own byte, so lanes stay independent under the 0x4040 mask.
        # bitVec ops keep in/out dtype equal (walrus rule) and the 16-bit
        # dtype enables the DVE 4x perf mode.
        return (shr, 1, band, 0x4040) if i == 7 else (shl, 6 - i, band, 0x4040)

    with tile.TileContext(nc) as tc:
        with (
            tc.tile_pool(name="consts", bufs=1) as cpool,
            tc.tile_pool(name="a2", bufs=a2_bufs) as a2pool,
            tc.tile_pool(name="w2", bufs=w2_bufs) as w2pool,
            tc.tile_pool(name="stage", bufs=stage_bufs) as spool,
            tc.tile_pool(name="outs", bufs=out_bufs) as opool,
            tc.tile_pool(name="psum", bufs=psum_bufs, space="PSUM") as ppool,
        ):
            c_rep = cpool.tile([P, O], f32, name="c_rep_t")
            r2t = cpool.tile([P, NB], f32, name="r2t_t")
            consts_loaded = False

            # repeat body for delta timing (rep>0 re-does identical work)
            for rep in range(reps):
              # ---- unpack activations (whole batch shard, kept resident) ----
              a2 = [
                  a2pool.tile([P, 2, B], u8, name=f"a2_{k2}")
                  for k2 in range(NK2)
              ]
              for h in range(NH):
                  # ---- unpack this phase's weight slice; in the first phase
                  # the activation unpack is interleaved (k2-major) so the PE
                  # can start as soon as the first a2/w2 pair lands ----
                  # w_blk: store each mm_width block's two pair-rows
                  # adjacently so the matmul rhs AP is fully contiguous
                  # (pair stride mm_width instead of OH)
                  w2_shape = (
                      [P, NOQ2, 2, mm_width] if w_blk else [P, 2, OH]
                  )
                  w2 = [
                      w2pool.tile(w2_shape, u8, name=f"w2_{k2}")
                      for k2 in range(NK2)
                  ]
                  for t in range(NT):
                      wt_st = spool.tile([P, OH], u8, name="wt_st")
                      nc.sync.dma_start(
                          out=wt_st,
                          in_=wt_d[t * P : (t + 1) * P, h * OH : (h + 1) * OH],
                      )
                      if h == 0 and host_a:
                          for q in range(4):
                              k2 = 4 * t + q
                              nc.sync.dma_start(
                                  out=a2[k2],
                                  in_=a2_d[k2 * P : (k2 + 1) * P, :],
                              )
                      elif h == 0:
                          at_st = spool.tile([P, B], u8, name="at_st")
                          nc.sync.dma_start(
                              out=at_st, in_=at_d[t * P : (t + 1) * P, :]
                          )
                      if not consts_loaded:
                          # emitted after the first stage DMAs: the epilogue
                          # constants (2 MiB) must not serialize the DMA
                          # queue ahead of the PE-critical first tiles
                          consts_loaded = True
                          nc.sync.dma_start(out=c_rep, in_=c_d)
                          nc.sync.dma_start(out=r2t, in_=r2_d)
                      for _ur in range(up_reps):
                        for i in range(8):
                          op0, s1, op1, s2 = unpack_ops(i)
                          if h == 0 and not host_a:
                              nc.vector.tensor_scalar(
                                  out=a2[4 * t + i // 2][:, i % 2, :].bitcast(u16),
                                  in0=at_st.bitcast(u16),
                                  scalar1=s1,
                                  scalar2=s2,
                                  op0=op0,
                                  op1=op1,
                              )
                          w2_out = (
                              w2[4 * t + i // 2][:, :, i % 2, :]
                              if w_blk
                              else w2[4 * t + i // 2][:, i % 2, :]
                          )
                          nc.vector.tensor_scalar(
                              out=w2_out.bitcast(u16),
                              in0=wt_st.bitcast(u16),
                              scalar1=s1,
                              scalar2=s2,
                              op0=op0,
                              op1=op1,
                          )

                  # ---- matmul + epilogue ----
                  NMM = OH // mm_width  # matmul instructions per (b, k2)
                  NPB = max(1, mm_width // 512)  # psum banks per matmul
                  for b in range(NB):
                      out_st = opool.tile([P, OH], f32, name="out_st")
                      psums = [
                          ppool.tile([P, mm_width], f32, name=f"ps_{om}")
                          for om in range(NMM)
                      ]
                      for _mr in range(mm_reps):
                        if mm_order == "k2":
                          # k2-major: one LDWEIGHTS per (b, k2), reused for
                          # NMM matmuls
                          for k2 in range(NK2):
                            lhsT = a2[k2][:, :, b * P : (b + 1) * P].bitcast(fp8)
                            for om in range(NMM):
                              rhs = (
                                  w2[k2][:, om, :, :]
                                  if w_blk
                                  else w2[k2][:, :, om * mm_width : (om + 1) * mm_width]
                              )
                              nc.tensor.matmul(
                                  psums[om],
                                  lhsT,
                                  rhs.bitcast(fp8),
                                  start=(k2 == 0),
                                  stop=(k2 == NK2 - 1),
                                  perf_mode=mybir.MatmulPerfMode.DoubleRow,
                              )
                        else:
                          # oq-major: lhsT changes EVERY matmul (LDWEIGHTS
                          # exposure probe)
                          for om in range(NMM):
                            for k2 in range(NK2):
                              rhs = (
                                  w2[k2][:, om, :, :]
                                  if w_blk
                                  else w2[k2][:, :, om * mm_width : (om + 1) * mm_width]
                              )
                              nc.tensor.matmul(
                                  psums[om],
                                  a2[k2][:, :, b * P : (b + 1) * P].bitcast(fp8),
                                  rhs.bitcast(fp8),
                                  start=(k2 == 0),
                                  stop=(k2 == NK2 - 1),
                                  perf_mode=mybir.MatmulPerfMode.DoubleRow,
                              )
                      for om in range(NMM):
                          osl = slice(om * mm_width, (om + 1) * mm_width)
                          csl = slice(
                              h * OH + om * mm_width,
                              h * OH + (om + 1) * mm_width,
                          )
                          nc.vector.scalar_tensor_tensor(
                              out=out_st[:, osl],
                              in0=psums[om],
                              scalar=r2t[:, b : b + 1],
                              in1=c_rep[:, csl],
                              op0=add,
                              op1=add,
                          )
                          # store each bank as soon as its epilogue lands so
                          # the final DMAs overlap the remaining epilogues
                          nc.sync.dma_start(
                              out=out_d[b * P : (b + 1) * P, csl],
                              in_=out_st[:, osl],
                          )

    nc.compile()
    return nc


_POP = np.unpackbits(np.arange(256, dtype=np.uint8)[:, None], axis=1).sum(1)


def _prep_inputs(input_packed, weight_packed, bias, B, O, K, n_cores,
                 host_a=False):
    """Host-side linear-time preprocessing: cast/transpose/shard + popcount
    rank-1 correction terms."""
    NB = B // n_cores // P
    A8 = input_packed.astype(np.uint8)  # [B, KP]
    W8 = weight_packed.astype(np.uint8)  # [O, KP]
    rA = _POP[A8].sum(1, dtype=np.int64)  # [B]
    rW = _POP[W8].sum(1, dtype=np.int64)  # [O]
    c = (bias.astype(np.float64) + K - 2.0 * rW).astype(np.float32)
    c_rep = np.ascontiguousarray(np.broadcast_to(c, (P, O)))
    r2 = (-2.0 * rA).astype(np.float32)
    at_all = np.ascontiguousarray(A8.T)  # [KP, B]
    wt = np.ascontiguousarray(W8.T)  # [KP, O]
    bsh = B // n_cores
    in_maps = []
    for ci in range(n_cores):
        sl = slice(ci * bsh, (ci + 1) * bsh)
        m = {
            "wt": wt,
            "c_rep": c_rep,
            "r2t": np.ascontiguousarray(r2[sl].reshape(NB, P).T),
        }
        at = np.ascontiguousarray(at_all[:, sl])  # [KP, bsh]
        if host_a:
            # unpack to {0, 0x40} bytes in the a2-tile layout:
            # row (4t+q)*P+p, col j*bsh+m = 0x40*bit_{2q+j}(at[t*P+p, m])
            KP = K // 8
            bits = (
                (at[:, :, None] >> np.arange(8, dtype=np.uint8)) & 1
            ).astype(np.uint8) * np.uint8(0x40)  # [KP, bsh, 8]
            V = bits.reshape(KP // P, P, bsh, 4, 2)  # [t, p, m, q, j]
            m["a2h"] = np.ascontiguousarray(
                V.transpose(0, 3, 1, 4, 2).reshape(4 * KP, 2 * bsh)
            )
        else:
            m["at"] = at
        in_maps.append(m)
    return in_maps


def kernel(input_packed, weight_packed, bias):
    global LAST_RESULTS
    from concourse.bass_utils import run_bass_kernel_spmd

    input_packed = np.asarray(input_packed)
    weight_packed = np.asarray(weight_packed)
    bias = np.asarray(bias)
    B, KP = input_packed.shape
    O = weight_packed.shape[0]
    K = KP * 8
    key = (B, O, K, N_CORES)
    if key not in _NC_CACHE:
        _NC_CACHE[key] = build_program(B // N_CORES, O, K, n_devices=N_CORES)
    nc = _NC_CACHE[key]

    in_maps = _prep_inputs(input_packed, weight_packed, bias, B, O, K, N_CORES)
    res = run_bass_kernel_spmd(nc, in_maps, list(range(N_CORES)))
    LAST_RESULTS = res
    out = np.concatenate([res.results[i]["out"] for i in range(N_CORES)], axis=0)
    return np.asarray(out, dtype=np.float32)

